# revision 1
# baseline (speedup 1.0000x reference)
"""Trainium2 Bass kernel for nn_MultiHeadAttention_54133767799241.

Full inputs -> full output. 8-core SPMD: data-parallel over batch (4) x
tensor-parallel over heads (2 groups of 8). Host folds the embedding
layer into the QKV projections (q = x @ (w_embed @ w_q) + b_embed @ w_q),
so the on-device contraction is 132 wide instead of 1024.

All q/k-path matmuls run as bf16 hi/lo split-3 accumulation
(a@b ~ ah@bh + ah@bl + al@bh, fp32 PSUM accumulate) -- ~16-17 effective
mantissa bits at bf16 speed (1 cyc/row, weight-load pull-ahead works,
unlike fp32 matmuls whose fused 4-byte weight load serializes).

Per-core pipeline (b = core//2, head group g = core%2, 8 heads as 4 pairs):
  1. qT/kT per pair [128=2x64d, 2048t] fp32 via split-3 (x and W splits
     precomputed on host); v natural [2048s, 512d] fp16
  2. on-device bf16 hi/lo split of q and k (truncation split via bitmask)
  3. scores pass 1: qh @ kh -> row max on DVE (only feeds the exp bias;
     +-2 logit error is harmless after renormalization)
  4. scores pass 2: split-3 into [128,1024] PSUM -> ACT exp(32S + bias)
     -> P fp16 + fused row sums (accum_out); 1/sum into P's col 2048
  5. P transposed via DMA xbar (fp16), incl. the recip column
  6. attn-out: v^T @ P^T fp16, A/B col-paired into separate PSUM banks;
     1/sum applied during PSUM->SBUF copyback (gpsimd-broadcast row)
  7. proj: oT @ w_proj slice -> partial y [2048, 136]
Host: y(b,0) + y(b,1) + b_proj.

Emission interleaves head-A / head-B matmuls so the PE runs them
concurrently in disjoint 64-row / 64-col groups of the systolic array.
"""
import sys

try:
    import concourse  # noqa: F401
except ImportError:
    sys.path.insert(0, "/opt/trn_rl_repo")

from contextlib import ExitStack

import ml_dtypes
import numpy as np

import concourse.bass as bass
import concourse.mybir as mybir
import concourse.tile as tile
from concourse import bacc
from concourse.bass_utils import run_bass_kernel_spmd

F32 = mybir.dt.float32
F16 = mybir.dt.float16
BF16 = mybir.dt.bfloat16
U32 = mybir.dt.uint32

T = 2048
NPAIR = 4
TTILES = 16
SCHUNKS = 4
EXP_SCALE = 32.0  # C**0.5 with C=1024 (faithful reference quirk)
OUT_DIM = 136

_cached = {}


def _build():
    nc = bacc.Bacc("TRN2", target_bir_lowering=False, debug=True)

    # x and the effective QKV weights arrive pre-split into bf16 hi/lo
    di = {}
    for nm, shape, dt in [
        ("xh_m", [128, T], BF16), ("xl_m", [128, T], BF16),
        ("xh_r", [4, T], BF16), ("xl_r", [4, T], BF16),
        ("wqh_m", [128, 512], BF16), ("wql_m", [128, 512], BF16),
        ("wqh_r", [4, 512], BF16), ("wql_r", [4, 512], BF16),
        ("wkh_m", [128, 512], BF16), ("wkl_m", [128, 512], BF16),
        ("wkh_r", [4, 512], BF16), ("wkl_r", [4, 512], BF16),
        ("wvh_m", [128, 512], BF16), ("wvl_m", [128, 512], BF16),
        ("wvh_r", [4, 512], BF16), ("wvl_r", [4, 512], BF16),
        ("bq", [128, 4], F32), ("bk", [128, 4], F32),
        ("bv", [128, 512], F32),
        ("wproj", [4, 128, OUT_DIM], F32),
    ]:
        di[nm] = nc.declare_dram_parameter(nm, shape, dt, isOutput=False)
    o_y = nc.declare_dram_parameter("y", [TTILES, 128, OUT_DIM], F32, isOutput=True)

    with tile.TileContext(nc) as tc, ExitStack() as ctx:
        const = ctx.enter_context(tc.tile_pool(name="const", bufs=1))
        qk_pool = ctx.enter_context(tc.tile_pool(name="qk", bufs=2))
        spl_pool = ctx.enter_context(tc.tile_pool(name="spl", bufs=1))
        scr_pool = ctx.enter_context(tc.tile_pool(name="scr", bufs=1))
        stat_pool = ctx.enter_context(tc.tile_pool(name="stat", bufs=8))
        pext_pool = ctx.enter_context(tc.tile_pool(name="pext", bufs=3))
        pt_pool = ctx.enter_context(tc.tile_pool(name="pt", bufs=1))
        ot_pool = ctx.enter_context(tc.tile_pool(name="ot", bufs=1))
        y_pool = ctx.enter_context(tc.tile_pool(name="ypool", bufs=4))
        # PSUM banks: psS 2 + ps2 4 + psA 1 + psB 1 = 8
        psS = ctx.enter_context(tc.tile_pool(name="psS", bufs=2, space="PSUM"))
        ps2 = ctx.enter_context(tc.tile_pool(name="ps2", bufs=4, space="PSUM"))
        psA = ctx.enter_context(tc.tile_pool(name="psA", bufs=1, space="PSUM"))
        psB = ctx.enter_context(tc.tile_pool(name="psB", bufs=1, space="PSUM"))

        tin = {}
        for nm, ap in di.items():
            if nm == "wproj":
                t = const.tile([128, 4, OUT_DIM], F32, name=f"t_{nm}")
                nc.sync.dma_start(t[:], ap.rearrange("c p e -> p c e"))
            else:
                t = const.tile(list(ap.shape), ap.dtype, name=f"t_{nm}")
                nc.sync.dma_start(t[:], ap[:])
            tin[nm] = t

        def six_mm(psum, lhs, rhs):
            """split-3 x (main+rem) accumulation into one psum region.
            lhs/rhs: (hi_main, hi_rem, lo_main, lo_rem) AP tuples."""
            lh_m, lh_r, ll_m, ll_r = lhs
            rh_m, rh_r, rl_m, rl_r = rhs
            seq = [(lh_m, rh_m), (lh_r, rh_r),
                   (lh_m, rl_m), (lh_r, rl_r),
                   (ll_m, rh_m), (ll_r, rh_r)]
            for i, (l, r) in enumerate(seq):
                nc.tensor.matmul(psum, l, r, start=(i == 0), stop=(i == len(seq) - 1))

        # ---- v natural [s, d] fp16 with bias ----
        t_v = const.tile([128, 16, 512], F16, name="t_v")
        for s in range(16):
            pv = psS.tile([128, 512], F32, tag="psS", name=f"pv{s}")
            sl = slice(s * 128, (s + 1) * 128)
            six_mm(pv[:],
                   (tin["xh_m"][:, sl], tin["xh_r"][:, sl],
                    tin["xl_m"][:, sl], tin["xl_r"][:, sl]),
                   (tin["wvh_m"][:], tin["wvh_r"][:],
                    tin["wvl_m"][:], tin["wvl_r"][:]))
            nc.vector.tensor_tensor(t_v[:, s, :], pv[:], tin["bv"][:],
                                    mybir.AluOpType.add)

        t_ot = ot_pool.tile([128, NPAIR, T], F32, name="t_ot")

        for m in range(NPAIR):
            msl = slice(m * 128, (m + 1) * 128)
            # ---- q/k fp32 for this pair ----
            t_qt = qk_pool.tile([128, T], F32, tag="qt", name=f"qt{m}")
            t_kt = qk_pool.tile([128, T], F32, tag="kt", name=f"kt{m}")
            for (wh, wl, whr, wlr, t_b, t_dst) in (
                    ("wqh_m", "wql_m", "wqh_r", "wql_r", "bq", t_qt),
                    ("wkh_m", "wkl_m", "wkh_r", "wkl_r", "bk", t_kt)):
                for tcb in range(SCHUNKS):
                    tsl = slice(tcb * 512, (tcb + 1) * 512)
                    pqk = psS.tile([128, 512], F32, tag="psS", name=f"pqk{m}{tcb}")
                    six_mm(pqk[:],
                           (tin[wh][:, msl], tin[whr][:, msl],
                            tin[wl][:, msl], tin[wlr][:, msl]),
                           (tin["xh_m"][:, tsl], tin["xh_r"][:, tsl],
                            tin["xl_m"][:, tsl], tin["xl_r"][:, tsl]))
                    nc.scalar.activation(t_dst[:, tsl], pqk[:],
                                         mybir.ActivationFunctionType.Identity,
                                         bias=tin[t_b][:, m:m + 1], scale=1.0)

            # ---- on-device hi/lo split of q, k (truncation split) ----
            t_qh = spl_pool.tile([128, T], BF16, tag="qh", name=f"qh{m}")
            t_ql = spl_pool.tile([128, T], BF16, tag="ql", name=f"ql{m}")
            t_kh = spl_pool.tile([128, T], BF16, tag="kh", name=f"kh{m}")
            t_kl = spl_pool.tile([128, T], BF16, tag="kl", name=f"kl{m}")
            for (t_src, t_hi, t_lo) in ((t_qt, t_qh, t_ql), (t_kt, t_kh, t_kl)):
                t_scr = scr_pool.tile([128, T], F32, tag="scr", name=f"scr{m}")
                nc.vector.tensor_scalar(t_scr[:].bitcast(U32), t_src[:].bitcast(U32),
                                        0xFFFF0000, None,
                                        mybir.AluOpType.bitwise_and)
                nc.vector.tensor_copy(t_hi[:], t_scr[:])
                nc.vector.tensor_tensor(t_lo[:], t_src[:], t_scr[:],
                                        mybir.AluOpType.subtract)

            for tc_i in range(SCHUNKS):  # t chunks of 512
                pt_A = pt_pool.tile([128, 17, 512], F16, tag="ptA", name=f"ptA{m}{tc_i}")
                pt_B = pt_pool.tile([128, 17, 512], F16, tag="ptB", name=f"ptB{m}{tc_i}")
                for tt in range(4):
                    t128 = tc_i * 4 + tt
                    tsl = slice(t128 * 128, (t128 + 1) * 128)
                    dsl = [slice(0, 64), slice(64, 128)]
                    pext = [pext_pool.tile([128, 2176], F16, tag="pext",
                                           name=f"pe{m}{t128}{h}") for h in range(2)]
                    mparts = [stat_pool.tile([128, 4], F32, tag=f"mp{h}",
                                             name=f"mp{m}{t128}{h}") for h in range(2)]
                    # pass 1: qh @ kh -> chunk maxes (A/B interleaved)
                    for sc in range(SCHUNKS):
                        ssl = slice(sc * 512, (sc + 1) * 512)
                        p1 = [psS.tile([128, 512], F32, tag="psS",
                                       name=f"p1{m}{t128}{h}{sc}") for h in range(2)]
                        for h in range(2):
                            nc.tensor.matmul(p1[h][:], t_qh[dsl[h], tsl],
                                             t_kh[dsl[h], ssl], start=True, stop=True)
                        for h in range(2):
                            nc.vector.tensor_reduce(mparts[h][:, sc:sc + 1], p1[h][:],
                                                    mybir.AxisListType.X,
                                                    mybir.AluOpType.max)
                    bias = []
                    for h in range(2):
                        t_mm = stat_pool.tile([128, 1], F32, tag=f"mm{h}",
                                              name=f"mx{m}{t128}{h}")
                        nc.vector.tensor_reduce(t_mm[:], mparts[h][:],
                                                mybir.AxisListType.X,
                                                mybir.AluOpType.max)
                        t_bi = stat_pool.tile([128, 1], F32, tag=f"bi{h}",
                                              name=f"bi{m}{t128}{h}")
                        nc.vector.tensor_scalar_mul(t_bi[:], t_mm[:], -EXP_SCALE)
                        bias.append(t_bi)
                    # pass 2: split-3 scores in [128,512] tiles -> exp -> P fp16
                    sums = [stat_pool.tile([128, 4], F32, tag=f"su{h}",
                                           name=f"su{m}{t128}{h}") for h in range(2)]
                    for sc in range(SCHUNKS):
                        csl = slice(sc * 512, (sc + 1) * 512)
                        p2 = [ps2.tile([128, 512], F32, tag="ps2",
                                       name=f"p2{m}{t128}{h}{sc}") for h in range(2)]
                        for trm, (lt, rt) in enumerate(
                                ((t_qh, t_kh), (t_qh, t_kl), (t_ql, t_kh))):
                            for h in range(2):
                                nc.tensor.matmul(
                                    p2[h][:], lt[dsl[h], tsl], rt[dsl[h], csl],
                                    start=(trm == 0), stop=(trm == 2))
                        for h in range(2):
                            nc.scalar.activation(
                                pext[h][:, csl], p2[h][:],
                                mybir.ActivationFunctionType.Exp,
                                bias=bias[h][:], scale=EXP_SCALE,
                                accum_out=sums[h][:, sc:sc + 1])
                    for h in range(2):
                        t_tot = stat_pool.tile([128, 1], F32, tag=f"to{h}",
                                               name=f"to{m}{t128}{h}")
                        nc.vector.tensor_reduce(t_tot[:], sums[h][:],
                                                mybir.AxisListType.X,
                                                mybir.AluOpType.add)
                        with nc.allow_low_precision(reason="1/sum stored fp16"):
                            nc.vector.reciprocal(pext[h][:, 2048:2049], t_tot[:])
                        nc.sync.dma_start_transpose(
                            (pt_A if h == 0 else pt_B)[:, :, tt * 128:(tt + 1) * 128],
                            pext[h][:])

                # ---- attn-out, col-paired into separate PSUM banks ----
                poA = psA.tile([128, 512], F32, tag="psA", name=f"poA{m}{tc_i}")
                poB = psB.tile([128, 512], F32, tag="psB", name=f"poB{m}{tc_i}")
                for si in range(16):
                    nc.tensor.matmul(poA[0:64, :], t_v[:, si, m * 128:m * 128 + 64],
                                     pt_A[:, si, :], start=(si == 0), stop=(si == 15))
                    nc.tensor.matmul(poB[64:128, :],
                                     t_v[:, si, m * 128 + 64:(m + 1) * 128],
                                     pt_B[:, si, :], start=(si == 0), stop=(si == 15),
                                     tile_position=(0, 64))
                csl = slice(tc_i * 512, (tc_i + 1) * 512)
                t_repA = stat_pool.tile([64, 512], F16, tag="repA", bufs=2,
                                        name=f"rA{m}{tc_i}")
                t_repB = stat_pool.tile([64, 512], F16, tag="repB", bufs=2,
                                        name=f"rB{m}{tc_i}")
                nc.gpsimd.partition_broadcast(t_repA[:], pt_A[0:1, 16, :])
                nc.gpsimd.partition_broadcast(t_repB[:], pt_B[0:1, 16, :])
                nc.vector.tensor_tensor(t_ot[0:64, m, csl], poA[0:64, :],
                                        t_repA[:], mybir.AluOpType.mult)
                nc.vector.tensor_tensor(t_ot[64:128, m, csl], poB[64:128, :],
                                        t_repB[:], mybir.AluOpType.mult)

        # ---- projection ----
        for t128 in range(TTILES):
            py = psS.tile([128, 512], F32, tag="psS", name=f"py{t128}")
            for mm_i in range(NPAIR):
                nc.tensor.matmul(py[:, 0:OUT_DIM],
                                 t_ot[:, mm_i, t128 * 128:(t128 + 1) * 128],
                                 tin["wproj"][:, mm_i, :],
                                 start=(mm_i == 0), stop=(mm_i == NPAIR - 1))
            t_y = y_pool.tile([128, OUT_DIM], F32, tag="y", name=f"y{t128}")
            nc.scalar.copy(t_y[:], py[:, 0:OUT_DIM])
            nc.sync.dma_start(o_y[t128], t_y[:])

    nc.finalize()
    return nc


def _bf16_split(a):
    hi = a.astype(ml_dtypes.bfloat16)
    lo = (a - hi.astype(np.float32)).astype(ml_dtypes.bfloat16)
    return hi.view(np.uint16), lo.view(np.uint16)


def _prep_group_inputs(w_embed, b_embed, w_q, w_k, w_v, w_proj):
    we64 = w_embed.astype(np.float64)
    be64 = b_embed.astype(np.float64)

    def eff(w):
        W = np.concatenate([we64 @ w[h].astype(np.float64) for h in range(w.shape[0])], axis=1)
        bias = np.concatenate([be64 @ w[h].astype(np.float64) for h in range(w.shape[0])])
        return W.astype(np.float32), bias.astype(np.float32)

    out = {}
    for nm, w in (("q", w_q), ("k", w_k), ("v", w_v)):
        W, b = eff(w)
        hi, lo = _bf16_split(W)
        out[f"w{nm}h_m"] = np.ascontiguousarray(hi[:128])
        out[f"w{nm}h_r"] = np.ascontiguousarray(hi[128:])
        out[f"w{nm}l_m"] = np.ascontiguousarray(lo[:128])
        out[f"w{nm}l_r"] = np.ascontiguousarray(lo[128:])
        if nm == "v":
            out["bv"] = np.ascontiguousarray(
                np.broadcast_to(b.reshape(1, 512), (128, 512)).astype(np.float32))
        else:
            out[f"b{nm}"] = np.ascontiguousarray(b.reshape(4, 128).T)
    out["wproj"] = np.ascontiguousarray(w_proj.reshape(4, 128, OUT_DIM))
    return out


def kernel(x, w_embed, b_embed, w_q, w_k, w_v, w_proj, b_proj):
    x = np.asarray(x, dtype=np.float32)
    w_embed = np.asarray(w_embed, dtype=np.float32)
    b_embed = np.asarray(b_embed, dtype=np.float32)
    w_q = np.asarray(w_q, dtype=np.float32)
    w_k = np.asarray(w_k, dtype=np.float32)
    w_v = np.asarray(w_v, dtype=np.float32)
    w_proj = np.asarray(w_proj, dtype=np.float32)
    b_proj = np.asarray(b_proj, dtype=np.float32)

    if "nc" not in _cached:
        _cached["nc"] = _build()
    nc = _cached["nc"]

    group_inputs = []
    for g in range(2):
        hsl = slice(g * 8, (g + 1) * 8)
        group_inputs.append(_prep_group_inputs(
            w_embed, b_embed, w_q[hsl], w_k[hsl], w_v[hsl],
            w_proj[g * 512:(g + 1) * 512]))

    in_maps = []
    core_ids = list(range(8))
    for c in core_ids:
        b, g = c // 2, c % 2
        xT = np.ascontiguousarray(x[b].T)
        xh, xl = _bf16_split(xT)
        im = dict(group_inputs[g])
        im["xh_m"] = np.ascontiguousarray(xh[:128])
        im["xh_r"] = np.ascontiguousarray(xh[128:])
        im["xl_m"] = np.ascontiguousarray(xl[:128])
        im["xl_r"] = np.ascontiguousarray(xl[128:])
        in_maps.append(im)

    rr = run_bass_kernel_spmd(nc, in_maps, core_ids)
    _cached["last"] = rr
    res = rr.results
    out = np.empty((4, T, OUT_DIM), dtype=np.float32)
    for b in range(4):
        y0 = np.asarray(res[2 * b]["y"]).reshape(T, OUT_DIM)
        y1 = np.asarray(res[2 * b + 1]["y"]).reshape(T, OUT_DIM)
        out[b] = y0 + y1 + b_proj
    return out

